# revision 1
# baseline (speedup 1.0000x reference)
"""Graph-transformer layer (GTLayer) on 8 Trainium2 NeuronCores.

Strategy (node-parallel, host-side edge binning as the sharding step):
  - Sort edges by destination node (row). Core c owns nodes
    [c*6250, (c+1)*6250) and receives exactly the edges pointing at its
    nodes, binned into 49 blocks of 128 destination nodes, each padded to
    T_B tiles of 128 edge slots (pad slots have local-id -1).
  - On device, each core computes k/v projection tables for ALL nodes
    (replicated work, no collectives) and a q table for its local nodes,
    then for each edge tile gathers q[row], k[col], v[col] via indirect
    DMA, computes per-head attention weights, and scatter-adds
    [weighted-v | exp-weight] into a per-block PSUM accumulator via a
    one-hot selection matmul (edges x local-node one-hot as lhsT).
  - Finalize per block: divide by (exp-sum + 1e-8), add residual,
    LayerNorm(eps=1e-6), write the block's 128 output rows.
  - Host concatenates the 8 per-core [6250, 128] outputs.

All cores run one identical program; per-core behavior differs only
through input data (binned index arrays + local embed slices).
"""

import numpy as np

import concourse.bass as bass
import concourse.bacc as bacc
import concourse.tile as tile
from concourse import mybir
from concourse.bass_utils import run_bass_kernel_spmd
from concourse.masks import make_identity

N = 50000
E = 800000
D = 128
H = 8
HD = 16
NCORES = 8
NPC = N // NCORES  # 6250 nodes per core
NB = (NPC + 127) // 128  # 49 blocks of 128 nodes per core
NBP = NB * 128  # 6272 padded local nodes
NPAD = ((N + 127) // 128) * 128  # 50048 padded table rows
NT = NPAD // 128  # 391 table blocks

F32 = mybir.dt.float32
I32 = mybir.dt.int32


def _bcast_inner(ap2d: bass.AP, k: int) -> bass.AP:
    """View a [P, m] AP as [P, m, k] with the inner dim broadcast (step 0)."""
    return bass.AP(tensor=ap2d.tensor, offset=ap2d.offset, ap=[*ap2d.ap, [0, k]])


def _head_view(ap2d: bass.AP) -> bass.AP:
    """View a [P, D] AP as [P, H, HD]."""
    return ap2d.rearrange("p (h x) -> p h x", h=H)


def build_program(t_b: int) -> bass.Bass:
    nc = bacc.Bacc(None, num_swdge_queues=4)

    embeds = nc.dram_tensor("embeds", [N, D], F32, kind="ExternalInput")
    emb_local = nc.dram_tensor("emb_local", [NBP, D], F32, kind="ExternalInput")
    qT = nc.dram_tensor("qT", [D, D], F32, kind="ExternalInput")
    kT = nc.dram_tensor("kT", [D, D], F32, kind="ExternalInput")
    vT = nc.dram_tensor("vT", [D, D], F32, kind="ExternalInput")
    lnsc = nc.dram_tensor("lnsc", [D], F32, kind="ExternalInput")
    lnb = nc.dram_tensor("lnb", [D], F32, kind="ExternalInput")
    lloc_d = nc.dram_tensor("lloc", [NB * 128, t_b], F32, kind="ExternalInput")
    qidx_d = nc.dram_tensor("qidx", [NB * 128, t_b], I32, kind="ExternalInput")
    cidx_d = nc.dram_tensor("cidx", [NB * 128, t_b], I32, kind="ExternalInput")

    kNodes = nc.dram_tensor("kNodes", [NPAD, D], F32)
    vNodes = nc.dram_tensor("vNodes", [NPAD, D], F32)
    qNodes = nc.dram_tensor("qNodes", [NBP, D], F32)

    out_d = nc.dram_tensor("out", [NBP, D], F32, kind="ExternalOutput")

    with tile.TileContext(nc) as tc:
        with tc.tile_pool(name="singles", bufs=1) as singles:
            # ---- one-time constants ----
            ident_g = singles.tile([128, 128], F32)
            make_identity(nc, ident_g)
            ident = singles.tile([128, 128], F32)
            nc.vector.tensor_copy(ident[:], ident_g[:])

            iota_i = singles.tile([128, 128], I32)
            nc.gpsimd.iota(iota_i[:], pattern=[[1, 128]], base=0, channel_multiplier=0)
            iota_f = singles.tile([128, 128], F32)
            nc.vector.tensor_copy(iota_f[:], iota_i[:])

            lnsc_t = singles.tile([128, 128], F32)
            nc.sync.dma_start(
                out=lnsc_t[:],
                in_=bass.AP(tensor=lnsc, offset=0, ap=[[0, 128], [1, 128]]),
            )
            lnb_t = singles.tile([128, 128], F32)
            nc.sync.dma_start(
                out=lnb_t[:],
                in_=bass.AP(tensor=lnb, offset=0, ap=[[0, 128], [1, 128]]),
            )
            eps_t = singles.tile([128, 1], F32)
            nc.vector.memset(eps_t[:], 1e-6)

            qT_t = singles.tile([128, 128], F32)
            nc.sync.dma_start(qT_t[:], qT[:])
            kT_t = singles.tile([128, 128], F32)
            nc.sync.dma_start(kT_t[:], kT[:])
            vT_t = singles.tile([128, 128], F32)
            nc.sync.dma_start(vT_t[:], vT[:])

            # ---- phase A: projection tables ----
            ctx_a = tc.tile_pool(name="tA", bufs=3)
            tA = ctx_a.__enter__()
            ctx_aps = tc.tile_pool(name="psA", bufs=2, space="PSUM")
            psA = ctx_aps.__enter__()
            for b in range(NT):
                sz = min(128, N - b * 128)
                emb_t = tA.tile([128, 128], F32)
                if sz < 128:
                    nc.vector.memset(emb_t[:], 0.0)
                nc.sync.dma_start(emb_t[:sz, :], embeds[b * 128 : b * 128 + sz, :])
                tp = psA.tile([128, 128], F32)
                nc.tensor.transpose(out=tp[:], in_=emb_t[:], identity=ident[:])
                embT = tA.tile([128, 128], F32)
                nc.scalar.copy(embT[:], tp[:])

                kp = psA.tile([128, 128], F32)
                nc.tensor.matmul(kp[:], lhsT=embT[:], rhs=kT_t[:], start=True, stop=True)
                ks = tA.tile([128, 128], F32)
                nc.vector.tensor_copy(ks[:], kp[:])
                nc.sync.dma_start(kNodes[b * 128 : (b + 1) * 128, :], ks[:])

                vp = psA.tile([128, 128], F32)
                nc.tensor.matmul(vp[:], lhsT=embT[:], rhs=vT_t[:], start=True, stop=True)
                vs = tA.tile([128, 128], F32)
                nc.vector.tensor_copy(vs[:], vp[:])
                nc.sync.dma_start(vNodes[b * 128 : (b + 1) * 128, :], vs[:])

            for b in range(NB):
                emb_t = tA.tile([128, 128], F32)
                nc.sync.dma_start(emb_t[:], emb_local[b * 128 : (b + 1) * 128, :])
                tp = psA.tile([128, 128], F32)
                nc.tensor.transpose(out=tp[:], in_=emb_t[:], identity=ident[:])
                embT = tA.tile([128, 128], F32)
                nc.scalar.copy(embT[:], tp[:])
                qp = psA.tile([128, 128], F32)
                nc.tensor.matmul(qp[:], lhsT=embT[:], rhs=qT_t[:], start=True, stop=True)
                qs = tA.tile([128, 128], F32)
                nc.vector.tensor_copy(qs[:], qp[:])
                nc.sync.dma_start(qNodes[b * 128 : (b + 1) * 128, :], qs[:])

            ctx_aps.__exit__(None, None, None)
            ctx_a.__exit__(None, None, None)

            # ---- phase B: edge tiles + scatter + finalize ----
            ctx_idx = tc.tile_pool(name="idxp", bufs=3)
            idxp = ctx_idx.__enter__()
            ctx_gat = tc.tile_pool(name="gat", bufs=12)
            gat = ctx_gat.__enter__()
            ctx_work = tc.tile_pool(name="work", bufs=8)
            work = ctx_work.__enter__()
            ctx_acc = tc.tile_pool(name="accps", bufs=4, space="PSUM")
            accps = ctx_acc.__enter__()
            ctx_fin = tc.tile_pool(name="finp", bufs=3)
            finp = ctx_fin.__enter__()
            for n in range(NB):
                r0 = n * 128
                lloc_t = idxp.tile([128, t_b], F32)
                nc.sync.dma_start(lloc_t[:], lloc_d[r0 : r0 + 128, :])
                qix_t = idxp.tile([128, t_b], I32)
                nc.sync.dma_start(qix_t[:], qidx_d[r0 : r0 + 128, :])
                cix_t = idxp.tile([128, t_b], I32)
                nc.sync.dma_start(cix_t[:], cidx_d[r0 : r0 + 128, :])

                acc = accps.tile([128, 136], F32)

                for t in range(t_b):
                    q_t = gat.tile([128, 128], F32)
                    nc.gpsimd.indirect_dma_start(
                        out=q_t[:],
                        out_offset=None,
                        in_=qNodes[:],
                        in_offset=bass.IndirectOffsetOnAxis(
                            ap=qix_t[:, t : t + 1], axis=0
                        ),
                    )
                    k_t = gat.tile([128, 128], F32)
                    nc.gpsimd.indirect_dma_start(
                        out=k_t[:],
                        out_offset=None,
                        in_=kNodes[:],
                        in_offset=bass.IndirectOffsetOnAxis(
                            ap=cix_t[:, t : t + 1], axis=0
                        ),
                    )
                    v_t = gat.tile([128, 128], F32)
                    nc.gpsimd.indirect_dma_start(
                        out=v_t[:],
                        out_offset=None,
                        in_=vNodes[:],
                        in_offset=bass.IndirectOffsetOnAxis(
                            ap=cix_t[:, t : t + 1], axis=0
                        ),
                    )

                    qk = work.tile([128, 128], F32)
                    nc.vector.tensor_tensor(
                        out=qk[:], in0=q_t[:], in1=k_t[:], op=mybir.AluOpType.mult
                    )
                    att = work.tile([128, H], F32)
                    nc.vector.tensor_reduce(
                        out=att[:],
                        in_=_head_view(qk[:]),
                        op=mybir.AluOpType.add,
                        axis=mybir.AxisListType.X,
                    )
                    attc = work.tile([128, H], F32)
                    nc.vector.tensor_scalar(
                        out=attc[:],
                        in0=att[:],
                        scalar1=10.0,
                        scalar2=-10.0,
                        op0=mybir.AluOpType.min,
                        op1=mybir.AluOpType.max,
                    )
                    expw = work.tile([128, H], F32)
                    nc.scalar.activation(
                        out=expw[:], in_=attc[:], func=mybir.ActivationFunctionType.Exp
                    )

                    x_t = work.tile([128, 136], F32)
                    nc.vector.tensor_tensor(
                        out=_head_view(x_t[:, 0:128]),
                        in0=_head_view(v_t[:]),
                        in1=_bcast_inner(expw[:], HD),
                        op=mybir.AluOpType.mult,
                    )
                    nc.gpsimd.tensor_copy(x_t[:, 128:136], expw[:])

                    p_t = work.tile([128, 128], F32)
                    nc.vector.tensor_scalar(
                        out=p_t[:],
                        in0=iota_f[:],
                        scalar1=lloc_t[:, t : t + 1],
                        scalar2=None,
                        op0=mybir.AluOpType.is_equal,
                    )

                    nc.tensor.matmul(
                        acc[:],
                        lhsT=p_t[:],
                        rhs=x_t[:],
                        start=(t == 0),
                        stop=(t == t_b - 1),
                    )

                # finalize block n
                accs = finp.tile([128, 136], F32)
                nc.vector.tensor_copy(accs[:], acc[:])
                dinv = finp.tile([128, H], F32)
                nc.vector.tensor_scalar_add(dinv[:], accs[:, 128:136], 1e-8)
                nc.vector.reciprocal(dinv[:], dinv[:])

                emb_t = finp.tile([128, 128], F32)
                nc.sync.dma_start(emb_t[:], emb_local[r0 : r0 + 128, :])

                res = finp.tile([128, 128], F32)
                nc.vector.tensor_tensor(
                    out=_head_view(res[:]),
                    in0=_head_view(accs[:, 0:128]),
                    in1=_bcast_inner(dinv[:], HD),
                    op=mybir.AluOpType.mult,
                )
                nc.vector.tensor_add(res[:], res[:], emb_t[:])

                stats = finp.tile([128, 6], F32)
                nc.vector.bn_stats(out=stats[:], in_=res[:])
                mv = finp.tile([128, 2], F32)
                nc.vector.bn_aggr(out=mv[:], in_=stats[:])

                sd = finp.tile([128, 1], F32)
                nc.scalar.activation(
                    out=sd[:],
                    in_=mv[:, 1:2],
                    func=mybir.ActivationFunctionType.Sqrt,
                    bias=eps_t[:],
                    scale=1.0,
                )
                nc.vector.reciprocal(sd[:], sd[:])

                xm = finp.tile([128, 128], F32)
                nc.vector.tensor_scalar_sub(xm[:], res[:], mv[:, 0:1])
                y = finp.tile([128, 128], F32)
                nc.vector.scalar_tensor_tensor(
                    out=y[:],
                    in0=xm[:],
                    scalar=sd[:],
                    in1=lnsc_t[:],
                    op0=mybir.AluOpType.mult,
                    op1=mybir.AluOpType.mult,
                )
                nc.vector.tensor_add(y[:], y[:], lnb_t[:])
                nc.sync.dma_start(out_d[r0 : r0 + 128, :], y[:])

            ctx_fin.__exit__(None, None, None)
            ctx_acc.__exit__(None, None, None)
            ctx_work.__exit__(None, None, None)
            ctx_gat.__exit__(None, None, None)
            ctx_idx.__exit__(None, None, None)

    nc.finalize()
    return nc


def _prepare_core_inputs(embeds, edge_index, qTrans, kTrans, vTrans, ln_scale, ln_bias):
    rows = np.asarray(edge_index[0]).astype(np.int64)
    cols = np.asarray(edge_index[1]).astype(np.int64)

    order = np.argsort(rows, kind="stable")
    rs = rows[order]
    cs = cols[order]

    core = rs // NPC
    local = rs - core * NPC
    blk = local >> 7
    lloc = (local & 127).astype(np.float32)
    g = core * NB + blk  # global block id, nondecreasing

    counts = np.bincount(g, minlength=NCORES * NB)
    t_b = max(2, int(np.ceil(counts.max() / 128)))
    cap = t_b * 128

    starts = np.zeros(NCORES * NB, dtype=np.int64)
    np.cumsum(counts[:-1], out=starts[1:])
    pos = np.arange(E, dtype=np.int64) - starts[g]
    slot = g * cap + pos

    nslots = NCORES * NB * cap
    lloc_a = np.full(nslots, -1.0, dtype=np.float32)
    qidx_a = np.zeros(nslots, dtype=np.int32)
    cidx_a = np.zeros(nslots, dtype=np.int32)
    lloc_a[slot] = lloc
    qidx_a[slot] = local.astype(np.int32)
    cidx_a[slot] = cs.astype(np.int32)

    # [ncores*NB, t_b, 128] -> [ncores, NB*128, t_b] so each block's
    # [128, t_b] SBUF tile is one contiguous DMA (partition p = edge lane,
    # column t = tile index).
    def to_tiles(a):
        a = a.reshape(NCORES, NB, t_b, 128).transpose(0, 1, 3, 2)
        return np.ascontiguousarray(a.reshape(NCORES, NB * 128, t_b))

    lloc_a = to_tiles(lloc_a)
    qidx_a = to_tiles(qidx_a)
    cidx_a = to_tiles(cidx_a)

    embeds = np.ascontiguousarray(np.asarray(embeds, dtype=np.float32))
    emb_pad = np.zeros((NCORES, NBP, D), dtype=np.float32)
    emb_pad[:, :NPC, :] = embeds.reshape(NCORES, NPC, D)

    qTrans = np.ascontiguousarray(np.asarray(qTrans, dtype=np.float32))
    kTrans = np.ascontiguousarray(np.asarray(kTrans, dtype=np.float32))
    vTrans = np.ascontiguousarray(np.asarray(vTrans, dtype=np.float32))
    ln_scale = np.ascontiguousarray(np.asarray(ln_scale, dtype=np.float32))
    ln_bias = np.ascontiguousarray(np.asarray(ln_bias, dtype=np.float32))

    in_maps = []
    for c in range(NCORES):
        in_maps.append(
            {
                "embeds": embeds,
                "emb_local": emb_pad[c],
                "qT": qTrans,
                "kT": kTrans,
                "vT": vTrans,
                "lnsc": ln_scale,
                "lnb": ln_bias,
                "lloc": lloc_a[c],
                "qidx": qidx_a[c],
                "cidx": cidx_a[c],
            }
        )
    return in_maps, t_b


_PROGRAM_CACHE: dict[int, bass.Bass] = {}


def kernel(embeds, edge_index, qTrans, kTrans, vTrans, ln_scale, ln_bias, **_):
    in_maps, t_b = _prepare_core_inputs(
        embeds, edge_index, qTrans, kTrans, vTrans, ln_scale, ln_bias
    )
    nc = _PROGRAM_CACHE.get(t_b)
    if nc is None:
        nc = build_program(t_b)
        _PROGRAM_CACHE[t_b] = nc

    res = run_bass_kernel_spmd(nc, in_maps, core_ids=list(range(NCORES)))
    outs = [res.results[c]["out"][:NPC] for c in range(NCORES)]
    return np.concatenate(outs, axis=0)


if __name__ == "__main__":
    rng = np.random.default_rng(0)
    inputs = {
        "embeds": rng.standard_normal((N, D), dtype=np.float32),
        "edge_index": rng.integers(0, N, size=(2, E)).astype(np.int64),
        "qTrans": (rng.standard_normal((D, D), dtype=np.float32) / np.sqrt(D)).astype(
            np.float32
        ),
        "kTrans": (rng.standard_normal((D, D), dtype=np.float32) / np.sqrt(D)).astype(
            np.float32
        ),
        "vTrans": (rng.standard_normal((D, D), dtype=np.float32) / np.sqrt(D)).astype(
            np.float32
        ),
        "ln_scale": np.ones(D, dtype=np.float32),
        "ln_bias": np.zeros(D, dtype=np.float32),
    }
    out = kernel(**inputs)
    print("kernel output", out.shape, out.dtype, np.isfinite(out).all())



# revision 8
# speedup vs baseline: 1.0834x; 1.0834x over previous
"""Graph-transformer layer (GTLayer) on 8 Trainium2 NeuronCores.

Strategy (node-parallel over destination nodes, degree-balanced binning):
  - Host: bin the 50000 nodes into 392 blocks of <=128 nodes with nearly
    equal total degree (snake deal over degree-sorted nodes), 49 blocks
    per core. Each block's edges are packed into t_b tiles of 128 edge
    slots (pad slots have lloc -1, dropped by the one-hot scatter).
  - Device, per core: compute q for its local nodes (49 blocks), then per
    block gather the block's edge data with per-tile indirect DMAs (raw
    col embeds + q rows), project k|v on the fly with one fused
    [128,256] fp32r matmul per 128-edge tile, compute per-head attention
    with 4-tile-wide DVE ops, and scatter-add [weighted-v | exp-weight]
    into a per-block PSUM accumulator via one-hot matmuls.
  - Finalize per block: divide by (exp-sum + 1e-8), add residual,
    LayerNorm(eps=1e-6), write the block's 128 output rows.
  - Host scatters the 8 per-core outputs back to original node order.

All cores run one identical program; per-core behavior differs only
through input data (binned index arrays + local embed slices).
"""

import numpy as np

import concourse.bass as bass
import concourse.bacc as bacc
import concourse.tile as tile
from concourse import mybir
from concourse.bass_utils import run_bass_kernel_spmd
from concourse.masks import make_identity

N = 50000
E = 800000
D = 128
H = 8
HD = 16
NCORES = 8
NB = 49  # blocks of 128 dest nodes per core
NBP = NB * 128  # padded local nodes per core
NBLK = NCORES * NB  # blocks total
BW = 4  # edge tiles per compute batch

F32 = mybir.dt.float32
F32R = mybir.dt.float32r
I32 = mybir.dt.int32


def _bcast_inner(ap: bass.AP, k: int) -> bass.AP:
    """View [..., m] AP as [..., m, k] with the inner dim broadcast."""
    return bass.AP(tensor=ap.tensor, offset=ap.offset, ap=[*ap.ap, [0, k]])


def build_program(t_b: int) -> bass.Bass:
    nc = bacc.Bacc(None, num_swdge_queues=4)

    embeds = nc.dram_tensor("embeds", [N, D], F32, kind="ExternalInput")
    emb_local = nc.dram_tensor("emb_local", [NBP, D], F32, kind="ExternalInput")
    qT = nc.dram_tensor("qT", [D, D], F32, kind="ExternalInput")
    kT = nc.dram_tensor("kT", [D, D], F32, kind="ExternalInput")
    vT = nc.dram_tensor("vT", [D, D], F32, kind="ExternalInput")
    lnsc = nc.dram_tensor("lnsc", [D], F32, kind="ExternalInput")
    lnb = nc.dram_tensor("lnb", [D], F32, kind="ExternalInput")
    lloc_d = nc.dram_tensor("lloc", [NBP, t_b], F32, kind="ExternalInput")
    qidx_d = nc.dram_tensor("qidx", [NBP, t_b], I32, kind="ExternalInput")
    cidx_d = nc.dram_tensor("cidx", [NBP, t_b], I32, kind="ExternalInput")

    qNodes = nc.dram_tensor("qNodes", [NBP, D], F32)
    out_d = nc.dram_tensor("out", [NBP, D], F32, kind="ExternalOutput")

    with tile.TileContext(nc) as tc:
        with tc.tile_pool(name="singles", bufs=1) as singles:
            # ---- one-time constants ----
            ident_g = singles.tile([128, 128], F32)
            make_identity(nc, ident_g)
            ident = singles.tile([128, 128], F32)
            nc.vector.tensor_copy(ident[:], ident_g[:])

            iota_i = singles.tile([128, BW, 128], I32)
            nc.gpsimd.iota(
                iota_i[:], pattern=[[0, BW], [1, 128]], base=0, channel_multiplier=0
            )
            iota_f = singles.tile([128, BW, 128], F32)
            nc.vector.tensor_copy(iota_f[:], iota_i[:])

            lnsc_t = singles.tile([128, 128], F32)
            nc.sync.dma_start(
                out=lnsc_t[:],
                in_=bass.AP(tensor=lnsc, offset=0, ap=[[0, 128], [1, 128]]),
            )
            lnb_t = singles.tile([128, 128], F32)
            nc.sync.dma_start(
                out=lnb_t[:],
                in_=bass.AP(tensor=lnb, offset=0, ap=[[0, 128], [1, 128]]),
            )
            eps_t = singles.tile([128, 1], F32)
            nc.vector.memset(eps_t[:], 1e-6)

            qT_t = singles.tile([128, 128], F32)
            nc.sync.dma_start(qT_t[:], qT[:])
            kvT_raw = singles.tile([128, 256], F32)
            nc.sync.dma_start(kvT_raw[:, 0:128], kT[:])
            nc.sync.dma_start(kvT_raw[:, 128:256], vT[:])
            kvT_t = singles.tile([128, 256], F32R)
            nc.vector.tensor_copy(kvT_t[:], kvT_raw[:])

            # ---- phase A: local q table ----
            with tc.tile_pool(name="tA", bufs=3) as tA, tc.tile_pool(
                name="psA", bufs=2, space="PSUM"
            ) as psA:
                for b in range(NB):
                    emb_t = tA.tile([128, 128], F32)
                    nc.sync.dma_start(emb_t[:], emb_local[b * 128 : (b + 1) * 128, :])
                    tp = psA.tile([128, 128], F32)
                    nc.tensor.transpose(out=tp[:], in_=emb_t[:], identity=ident[:])
                    embT = tA.tile([128, 128], F32)
                    nc.scalar.copy(embT[:], tp[:])
                    qp = psA.tile([128, 128], F32)
                    nc.tensor.matmul(
                        qp[:], lhsT=embT[:], rhs=qT_t[:], start=True, stop=True
                    )
                    qs = tA.tile([128, 128], F32)
                    nc.vector.tensor_copy(qs[:], qp[:])
                    nc.sync.dma_start(qNodes[b * 128 : (b + 1) * 128, :], qs[:])

            # ---- phase B: per block gather + attention + scatter ----
            with tc.tile_pool(name="idxp", bufs=3) as idxp, tc.tile_pool(
                name="gat", bufs=2
            ) as gat, tc.tile_pool(name="work", bufs=3) as work, tc.tile_pool(
                name="tpps", bufs=2, space="PSUM"
            ) as tpps, tc.tile_pool(
                name="kvps", bufs=2, space="PSUM"
            ) as kvps, tc.tile_pool(
                name="accps", bufs=2, space="PSUM"
            ) as accps, tc.tile_pool(name="finp", bufs=2) as finp:
                for n in range(NB):
                    r0 = n * 128
                    lloc_t = idxp.tile([128, t_b], F32)
                    nc.sync.dma_start(lloc_t[:], lloc_d[r0 : r0 + 128, :])
                    qix_t = idxp.tile([128, t_b], I32)
                    nc.sync.dma_start(qix_t[:], qidx_d[r0 : r0 + 128, :])
                    cix_t = idxp.tile([128, t_b], I32)
                    nc.sync.dma_start(cix_t[:], cidx_d[r0 : r0 + 128, :])

                    e_all = gat.tile([128, t_b, 128], F32)
                    q_all = gat.tile([128, t_b, 128], F32)
                    for t in range(t_b):
                        nc.gpsimd.indirect_dma_start(
                            out=e_all[:, t, :],
                            out_offset=None,
                            in_=embeds[:],
                            in_offset=bass.IndirectOffsetOnAxis(
                                ap=cix_t[:, t : t + 1], axis=0
                            ),
                        )
                        nc.gpsimd.indirect_dma_start(
                            out=q_all[:, t, :],
                            out_offset=None,
                            in_=qNodes[:],
                            in_offset=bass.IndirectOffsetOnAxis(
                                ap=qix_t[:, t : t + 1], axis=0
                            ),
                        )

                    acc = accps.tile([128, 136], F32)

                    for t0 in range(0, t_b, BW):
                        w = min(BW, t_b - t0)
                        tp4 = tpps.tile([128, BW, 128], F32)
                        for j in range(w):
                            nc.tensor.transpose(
                                out=tp4[:, j, :],
                                in_=e_all[:, t0 + j, :],
                                identity=ident[:],
                            )
                        ect = work.tile([128, BW, 128], F32R)
                        nc.scalar.copy(ect[:, 0:w, :], tp4[:, 0:w, :])

                        kv = kvps.tile([128, BW, 256], F32)
                        for j in range(w):
                            nc.tensor.matmul(
                                kv[:, j, :],
                                lhsT=ect[:, j, :],
                                rhs=kvT_t[:],
                                start=True,
                                stop=True,
                                skip_group_check=True,
                            )

                        qk = work.tile([128, BW, 128], F32)
                        nc.vector.tensor_tensor(
                            out=qk[:, 0:w, :],
                            in0=q_all[:, t0 : t0 + w, :],
                            in1=kv[:, 0:w, 0:128],
                            op=mybir.AluOpType.mult,
                        )
                        att = work.tile([128, BW * H], F32)
                        nc.vector.tensor_reduce(
                            out=att[:, 0 : w * H],
                            in_=qk[:, 0:w, :].rearrange("p w (h x) -> p (w h) x", h=H),
                            op=mybir.AluOpType.add,
                            axis=mybir.AxisListType.X,
                        )
                        attc = work.tile([128, BW * H], F32)
                        nc.gpsimd.tensor_scalar(
                            out=attc[:, 0 : w * H],
                            in0=att[:, 0 : w * H],
                            scalar1=10.0,
                            scalar2=-10.0,
                            op0=mybir.AluOpType.min,
                            op1=mybir.AluOpType.max,
                        )
                        expw = work.tile([128, BW * H], F32)
                        nc.scalar.activation(
                            out=expw[:, 0 : w * H],
                            in_=attc[:, 0 : w * H],
                            func=mybir.ActivationFunctionType.Exp,
                        )

                        x_t = work.tile([128, BW, 136], F32)
                        nc.vector.tensor_tensor(
                            out=x_t[:, 0:w, 0:128].rearrange(
                                "p w (h x) -> p w h x", h=H
                            ),
                            in0=kv[:, 0:w, 128:256].rearrange(
                                "p w (h x) -> p w h x", h=H
                            ),
                            in1=_bcast_inner(
                                expw[:, 0 : w * H].rearrange("p (w h) -> p w h", h=H),
                                HD,
                            ),
                            op=mybir.AluOpType.mult,
                        )
                        nc.gpsimd.tensor_copy(
                            x_t[:, 0:w, 128:136],
                            expw[:, 0 : w * H].rearrange("p (w h) -> p w h", h=H),
                        )

                        p_t = work.tile([128, BW, 128], F32)
                        nc.vector.tensor_tensor(
                            out=p_t[:, 0:w, :],
                            in0=iota_f[:, 0:w, :],
                            in1=_bcast_inner(lloc_t[:, t0 : t0 + w], 128),
                            op=mybir.AluOpType.is_equal,
                        )

                        for j in range(w):
                            nc.tensor.matmul(
                                acc[:],
                                lhsT=p_t[:, j, :],
                                rhs=x_t[:, j, :],
                                start=(t0 + j == 0),
                                stop=(t0 + j == t_b - 1),
                                skip_group_check=True,
                            )

                    # finalize block n
                    accs = finp.tile([128, 136], F32)
                    nc.vector.tensor_copy(accs[:], acc[:])
                    dinv = finp.tile([128, H], F32)
                    nc.vector.tensor_scalar_add(dinv[:], accs[:, 128:136], 1e-8)
                    nc.vector.reciprocal(dinv[:], dinv[:])

                    emb_t = finp.tile([128, 128], F32)
                    nc.sync.dma_start(emb_t[:], emb_local[r0 : r0 + 128, :])

                    res = finp.tile([128, 128], F32)
                    nc.vector.tensor_tensor(
                        out=res[:].rearrange("p (h x) -> p h x", h=H),
                        in0=accs[:, 0:128].rearrange("p (h x) -> p h x", h=H),
                        in1=_bcast_inner(dinv[:], HD),
                        op=mybir.AluOpType.mult,
                    )
                    nc.vector.tensor_add(res[:], res[:], emb_t[:])

                    stats = finp.tile([128, 6], F32)
                    nc.vector.bn_stats(out=stats[:], in_=res[:])
                    mv = finp.tile([128, 2], F32)
                    nc.vector.bn_aggr(out=mv[:], in_=stats[:])

                    sd = finp.tile([128, 1], F32)
                    nc.scalar.activation(
                        out=sd[:],
                        in_=mv[:, 1:2],
                        func=mybir.ActivationFunctionType.Sqrt,
                        bias=eps_t[:],
                        scale=1.0,
                    )
                    nc.vector.reciprocal(sd[:], sd[:])

                    xm = finp.tile([128, 128], F32)
                    nc.vector.tensor_scalar_sub(xm[:], res[:], mv[:, 0:1])
                    y = finp.tile([128, 128], F32)
                    nc.vector.scalar_tensor_tensor(
                        out=y[:],
                        in0=xm[:],
                        scalar=sd[:],
                        in1=lnsc_t[:],
                        op0=mybir.AluOpType.mult,
                        op1=mybir.AluOpType.mult,
                    )
                    nc.vector.tensor_add(y[:], y[:], lnb_t[:])
                    nc.sync.dma_start(out_d[r0 : r0 + 128, :], y[:])

    nc.finalize()
    return nc


def _prepare_core_inputs(embeds, edge_index, qTrans, kTrans, vTrans, ln_scale, ln_bias):
    rows = np.asarray(edge_index[0]).astype(np.int64)
    cols = np.asarray(edge_index[1]).astype(np.int64)
    n_nodes = N

    # --- degree-balanced node->block binning (snake deal) ---
    deg = np.bincount(rows, minlength=n_nodes)
    order = np.argsort(-deg, kind="stable")
    idx = np.arange(n_nodes, dtype=np.int64)
    rnd = idx // NBLK
    pos = idx % NBLK
    snake = np.where(rnd % 2 == 0, pos, NBLK - 1 - pos)
    blk = np.empty(n_nodes, dtype=np.int64)
    slot = np.empty(n_nodes, dtype=np.int64)
    blk[order] = snake
    slot[order] = rnd
    gslot = blk * 128 + slot  # node -> padded global row

    # --- group edges by destination block ---
    be = blk[rows]
    order_e = np.argsort(be, kind="stable")
    be_s = be[order_e]
    lloc_s = slot[rows[order_e]].astype(np.float32)
    qidx_s = ((be_s % NB) * 128 + slot[rows[order_e]]).astype(np.int32)
    cidx_s = cols[order_e].astype(np.int32)

    counts = np.bincount(be_s, minlength=NBLK)
    t_b = max(2, int(np.ceil(counts.max() / 128)))
    cap = t_b * 128

    starts = np.zeros(NBLK, dtype=np.int64)
    np.cumsum(counts[:-1], out=starts[1:])
    pos_e = np.arange(E, dtype=np.int64) - starts[be_s]
    slot_e = be_s * cap + pos_e

    nslots = NBLK * cap
    lloc_a = np.full(nslots, -1.0, dtype=np.float32)
    qidx_a = np.zeros(nslots, dtype=np.int32)
    cidx_a = np.zeros(nslots, dtype=np.int32)
    lloc_a[slot_e] = lloc_s
    qidx_a[slot_e] = qidx_s
    cidx_a[slot_e] = cidx_s

    # [NBLK, t_b, 128] -> [ncores, NB*128, t_b]: partition p = edge lane,
    # column t = tile index, so each block's [128, t_b] tile is one DMA.
    def to_tiles(a):
        a = a.reshape(NCORES, NB, t_b, 128).transpose(0, 1, 3, 2)
        return np.ascontiguousarray(a.reshape(NCORES, NBP, t_b))

    lloc_a = to_tiles(lloc_a)
    qidx_a = to_tiles(qidx_a)
    cidx_a = to_tiles(cidx_a)

    embeds = np.ascontiguousarray(np.asarray(embeds, dtype=np.float32))
    emb_pad = np.zeros((NCORES * NBP, D), dtype=np.float32)
    emb_pad[gslot] = embeds
    emb_pad = emb_pad.reshape(NCORES, NBP, D)

    qTrans = np.ascontiguousarray(np.asarray(qTrans, dtype=np.float32))
    kTrans = np.ascontiguousarray(np.asarray(kTrans, dtype=np.float32))
    vTrans = np.ascontiguousarray(np.asarray(vTrans, dtype=np.float32))
    ln_scale = np.ascontiguousarray(np.asarray(ln_scale, dtype=np.float32))
    ln_bias = np.ascontiguousarray(np.asarray(ln_bias, dtype=np.float32))

    in_maps = []
    for c in range(NCORES):
        in_maps.append(
            {
                "embeds": embeds,
                "emb_local": emb_pad[c],
                "qT": qTrans,
                "kT": kTrans,
                "vT": vTrans,
                "lnsc": ln_scale,
                "lnb": ln_bias,
                "lloc": lloc_a[c],
                "qidx": qidx_a[c],
                "cidx": cidx_a[c],
            }
        )
    return in_maps, t_b, gslot


_PROGRAM_CACHE: dict[int, bass.Bass] = {}


def kernel(embeds, edge_index, qTrans, kTrans, vTrans, ln_scale, ln_bias, **_):
    in_maps, t_b, gslot = _prepare_core_inputs(
        embeds, edge_index, qTrans, kTrans, vTrans, ln_scale, ln_bias
    )
    nc = _PROGRAM_CACHE.get(t_b)
    if nc is None:
        nc = build_program(t_b)
        _PROGRAM_CACHE[t_b] = nc

    res = run_bass_kernel_spmd(nc, in_maps, core_ids=list(range(NCORES)))
    all_out = np.concatenate([res.results[c]["out"] for c in range(NCORES)], axis=0)
    return np.ascontiguousarray(all_out[gslot])


if __name__ == "__main__":
    rng = np.random.default_rng(0)
    inputs = {
        "embeds": rng.standard_normal((N, D), dtype=np.float32),
        "edge_index": rng.integers(0, N, size=(2, E)).astype(np.int64),
        "qTrans": (rng.standard_normal((D, D), dtype=np.float32) / np.sqrt(D)).astype(
            np.float32
        ),
        "kTrans": (rng.standard_normal((D, D), dtype=np.float32) / np.sqrt(D)).astype(
            np.float32
        ),
        "vTrans": (rng.standard_normal((D, D), dtype=np.float32) / np.sqrt(D)).astype(
            np.float32
        ),
        "ln_scale": np.ones(D, dtype=np.float32),
        "ln_bias": np.zeros(D, dtype=np.float32),
    }
    out = kernel(**inputs)
    print("kernel output", out.shape, out.dtype, np.isfinite(out).all())
